# revision 1
# baseline (speedup 1.0000x reference)
"""Trainium2 Bass kernel for nn_DynamicWeightedMSELoss.

loss = mean(w * (input-target)^2), w = 1 - counts[c][k]/total_c when the
rounded value round(10*x)/10 hits histogram bin k of channel c, else 1.

Strategy (pure data parallel over 8 cores, batch axis):
  loss*N = sum(d^2) - sum(q[k]*d^2),  q = cnt/total in [0, ~0.01]
  - per element: k-index = clamp(round(10x), -101, 101)+101+203*c via
    DVE magic-number rounding (exact round-half-even) -> int16 index
  - per-element table lookup via GPSIMD ap_gather from a replicated
    [128, 1016] combined q-table (5 channels x 203 entries incl. OOR
    sentinels)
  - ap_gather uses one shared index list per 16-partition core group in
    (s p)-wrapped order; we therefore load x a second time through a
    permuted DMA access pattern ("B layout") chosen so that the gather
    output de-interleaves with contiguous 2.5KB runs back into the
    natural layout ("A layout") where d^2 lives.
  - reductions: per-instruction accum_out -> [128, nchunk] strips ->
    [128, 2] per-core partials; final scalar combine on host.

The q-table is built on the host from the actual steps/counts inputs by
mirroring the reference searchsorted+match logic for every possible
rounded value m/10, m in [-101, 101] (fp32 arithmetic mirrored exactly).
"""

import numpy as np

import concourse.bacc as bacc
import concourse.bass as bass
import concourse.mybir as mybir
import concourse.tile as tile
from concourse.bass_utils import run_bass_kernel_spmd

F32 = mybir.dt.float32
I16 = mybir.dt.int16
ALU = mybir.AluOpType
ACTF = mybir.ActivationFunctionType

N_CORES = 8
B_FULL = 4194304
CH = 5
NBINS = 201
TBL = 203 * CH + 1  # 1016, padded combined table size
MAGIC = 12582912.0  # 1.5 * 2**23: float32 round-to-nearest-even trick

P = 128


def _host_qtable(steps: np.ndarray, counts: np.ndarray) -> np.ndarray:
    """Mirror of the reference weight logic for every rounded value.

    Returns combined q table [1016] f32: entry 203*c + (m+101) = cnt/total
    for the bin matching rounded value m/10 (f32), else 0.0 (weight 1).
    """
    steps = np.asarray(steps, np.float32)
    counts_f = np.asarray(counts, np.float32)
    totals = counts_f.sum(axis=1, dtype=np.float32)  # integer-valued, exact
    tab = np.zeros(TBL, np.float32)
    ms = np.arange(-101, 102, dtype=np.float32)
    for c in range(CH):
        r = (ms / np.float32(10.0)).astype(np.float32)  # fl(m/10)
        idx = np.searchsorted(steps[c], r).clip(0, steps.shape[1] - 1)
        match = np.abs(steps[c][idx] - r) < 1e-4
        q = np.where(match, counts_f[c][idx] / totals[c], np.float32(0.0))
        tab[203 * c : 203 * c + 203] = q.astype(np.float32)
    return tab


def build_nc(
    rows_per_core: int,
    T: int = 640,
    *,
    skip_gather: bool = False,
    skip_rearrange_use_gout_row: bool = False,
    reps: int = 1,
    n_cores: int = N_CORES,
):
    """Emit + compile the per-core SPMD program.

    T must be a multiple of 80 (16 for the B layout wrap, 5 so the
    channel pattern is tile-invariant). reps>1 reruns the whole chunk
    loop for wall-clock timing (results then invalid).
    """
    assert T % 80 == 0
    flat = rows_per_core * CH
    assert flat % (P * T) == 0
    nchunk = flat // (P * T)
    cols = flat // P  # per-partition elements
    U = T // 16

    # detect_race_conditions=False: the simulator's race detector handles
    # partition-strided slice reads (gout[r::16, ...]) conservatively and
    # false-positives against adjacent tiles; the generated waits were
    # hand-verified and results are checked against hardware.
    nc = bacc.Bacc(
        "TRN2",
        target_bir_lowering=False,
        debug=False,
        num_devices=n_cores,
        detect_race_conditions=False,
    )

    x_d = nc.dram_tensor("x", [flat], F32, kind="ExternalInput")
    t_d = nc.dram_tensor("t", [flat], F32, kind="ExternalInput")
    xw_d = nc.dram_tensor("xw", [flat], F32, kind="ExternalInput")
    tab_d = nc.dram_tensor("tab", [P, TBL], F32, kind="ExternalInput")
    cho_d = nc.dram_tensor("cho", [P, T], F32, kind="ExternalInput")
    out_d = nc.dram_tensor("out", [P, 2], F32, kind="ExternalOutput")

    # A layout: natural row-contiguous [chunk, 128, T]
    xa_v = x_d.ap().rearrange("(p t s) -> t p s", p=P, t=nchunk, s=T)
    ta_v = t_d.ap().rearrange("(p t s) -> t p s", p=P, t=nchunk, s=T)
    # B (wrapped) layout comes from the host-permuted copy xw, so its
    # load is a plain contiguous tile load.
    xb_v = xw_d.ap().rearrange("(p t s) -> t p s", p=P, t=nchunk, s=T)

    from contextlib import ExitStack
    with tile.TileContext(nc) as tc, ExitStack() as stk:
        cpool = stk.enter_context(tc.tile_pool(name="const", bufs=1))
        tab = cpool.tile([P, TBL], F32)
        nc.sync.dma_start(tab[:], tab_d.ap())
        cho = cpool.tile([P, T], F32)
        nc.sync.dma_start(cho[:], cho_d.ap())
        sd2_strip = cpool.tile([P, nchunk], F32, tag="sd2s")
        qd2_strip = cpool.tile([P, nchunk], F32, tag="qd2s")

        pool = stk.enter_context(tc.tile_pool(name="work", bufs=3))
        # persistent ping-pong gather-output tiles: partition-strided slice
        # reads of pool-cycled tiles confuse the aliasing checker, and a
        # stable address keeps dependency tracking exact.
        gpool = stk.enter_context(tc.tile_pool(name="gout", bufs=1))
        gouts = []
        for i in range(2):
            g_tile = gpool.tile([P, 16 * T], F32, tag=f"gout{i}", name=f"gout{i}")
            gouts.append(g_tile)

        for _ in range(reps):
            for ti in range(nchunk):
                xa = pool.tile([P, T], F32, tag="xa")
                nc.sync.dma_start(xa[:], xa_v[ti])
                ta = pool.tile([P, T], F32, tag="ta")
                nc.sync.dma_start(ta[:], ta_v[ti])
                xb = pool.tile([P, T], F32, tag="xb")
                nc.sync.dma_start(xb[:], xb_v[ti])

                # index pipeline (from B layout)
                y = pool.tile([P, T], F32, tag="y")
                nc.scalar.mul(y[:], xb[:], 10.0)  # ACT: fl(10x)
                mm = pool.tile([P, T], F32, tag="mm")
                # mm = min(round(10x)+MAGIC, MAGIC+101)
                nc.vector.tensor_scalar(
                    mm[:], y[:], MAGIC, MAGIC + 101.0, ALU.add, ALU.min
                )
                idx = pool.tile([P, T], I16, tag="idx")
                # idx = max(mm, MAGIC-101) + (203*c + 101 - MAGIC)
                nc.vector.scalar_tensor_tensor(
                    idx[:], mm[:], MAGIC - 101.0, cho[:], ALU.max, ALU.add
                )

                # d^2 pipeline (A layout)
                d = pool.tile([P, T], F32, tag="d")
                nc.vector.tensor_tensor(d[:], xa[:], ta[:], ALU.subtract)
                d2 = pool.tile([P, T], F32, tag="d2")
                nc.scalar.activation(
                    d2[:], d[:], ACTF.Square,
                    accum_out=sd2_strip[:, ti : ti + 1],
                )

                # gather q = tab[idx] (per 16-partition group shared idx)
                gout = gouts[ti % 2]
                if not skip_gather:
                    nc.gpsimd.ap_gather(
                        gout[:], tab[:], idx[:],
                        channels=P, num_elems=TBL, d=1, num_idxs=16 * T,
                    )
                # de-interleave using replication: w2[16j+r, s] =
                # gout[16j+r, T*r + s]; 16 clean [8, T] slice DMAs.
                w2 = pool.tile([P, T], F32, tag="w2")
                if skip_rearrange_use_gout_row:
                    nc.vector.tensor_copy(w2[:], gout[:, :T])
                else:
                    for r in range(16):
                        nc.sync.dma_start(
                            w2[r::16, :], gout[r::16, T * r : T * r + T]
                        )

                prod = pool.tile([P, T], F32, tag="prod")
                nc.vector.scalar_tensor_tensor(
                    prod[:], w2[:], 0.0, d2[:], ALU.bypass, ALU.mult,
                    accum_out=qd2_strip[:, ti : ti + 1],
                )

        res = cpool.tile([P, 2], F32, tag="res")
        nc.vector.tensor_reduce(res[:, 0:1], sd2_strip[:], mybir.AxisListType.X, ALU.add)
        nc.vector.tensor_reduce(res[:, 1:2], qd2_strip[:], mybir.AxisListType.X, ALU.add)
        nc.sync.dma_start(out_d.ap(), res[:])

    nc.compile()
    return nc


def _wrap_permute(core_slice: np.ndarray, T: int) -> np.ndarray:
    """Host-side permutation producing the wrapped ("B") layout copy.

    xw[(16j+a)*cols + T*t + U*r + u] = x[(16j+r)*cols + T*t + 16*u + a]
    so the device's wrapped load is a plain contiguous tile load.
    """
    flat = core_slice.size
    cols = flat // P
    nchunk = cols // T
    U = T // 16
    v = core_slice.reshape(P // 16, 16, nchunk, U, 16)  # j, r, t, u, a
    return np.ascontiguousarray(np.transpose(v, (0, 4, 2, 1, 3))).reshape(-1)


def _host_inputs(input, target, steps, counts, rows_per_core, T, n_cores=N_CORES):
    flat_all = np.ascontiguousarray(input, dtype=np.float32).reshape(-1)
    tflat_all = np.ascontiguousarray(target, dtype=np.float32).reshape(-1)
    tab = _host_qtable(steps, counts)
    tab_full = np.broadcast_to(tab, (P, TBL)).copy()
    # choff[p, m] = 203*((u + a) mod 5) + 101 - MAGIC; a = p%16, u = m%U
    U = T // 16
    a = (np.arange(P) % 16)[:, None]
    u = (np.arange(T) % U)[None, :]
    cho = (203.0 * ((u + a) % CH) + 101.0 - MAGIC).astype(np.float32)
    flat_pc = rows_per_core * CH
    in_maps = []
    for c in range(n_cores):
        xs = flat_all[c * flat_pc : (c + 1) * flat_pc]
        in_maps.append(
            {
                "x": xs,
                "t": tflat_all[c * flat_pc : (c + 1) * flat_pc],
                "xw": _wrap_permute(xs, T),
                "tab": tab_full,
                "cho": cho,
            }
        )
    return in_maps


_NC_CACHE: dict = {}


def kernel(input, target, steps, counts):
    rows_per_core = B_FULL // N_CORES
    T = 640
    key = (rows_per_core, T)
    if key not in _NC_CACHE:
        _NC_CACHE[key] = build_nc(rows_per_core, T)
    nc = _NC_CACHE[key]
    in_maps = _host_inputs(input, target, steps, counts, rows_per_core, T)
    res = run_bass_kernel_spmd(nc, in_maps, core_ids=list(range(N_CORES)))
    total = np.float64(0.0)
    for c in range(N_CORES):
        part = res.results[c]["out"].astype(np.float64)  # [128, 2]
        total += part[:, 0].sum() - part[:, 1].sum()
    n = np.float64(B_FULL * CH)
    return np.float32(total / n)


def numpy_model_partials(in_maps):
    """Reference partials for the bass kernel (per core [128,2] sums)."""
    outs = []
    for m in in_maps:
        x = m["x"].astype(np.float64)
        t = m["t"].astype(np.float64)
        tab = m["tab"][0].astype(np.float64)
        y = np.float32(10.0) * m["x"].astype(np.float32)
        k = np.clip(np.round(y.astype(np.float64)), -101, 101).astype(np.int64)
        flatidx = np.arange(x.size)
        c = flatidx % CH
        idx = 203 * c + k + 101
        q = tab[idx]
        d2 = (x - t) ** 2
        outs.append((d2.sum(), (q * d2).sum()))
    return outs

